# revision 29
# baseline (speedup 1.0000x reference)
"""Trainium2 Bass kernel for AverageSpanExtractor (segment mean over spans).

Math note: the reference's masked softmax over all-ones logits reduces
exactly to a mean over the span tokens [start, end):
    out[b, n, :] = mean(sequence_tensor[b, start:end, :]).

Strategy (8 cores, batch-parallel — one batch element per core), built
around sorted-span segment matmuls. Indexed-fetch approaches (SWDGE
gather, ap_gather, indirect_copy) all bottom out at >=20us for the 3k
random row fetches this problem needs; the PE is power-throttled to
~1.2 GHz with ~180ns fixed cost per matmul, so the design minimizes PE
instruction count:

  1. HOST: sort each batch's spans by start. A 128-span chunk of the
     sorted order covers a ~640-token window (5-7 of the 32 128-token
     blocks). Window bounds are unioned across the 8 cores so one SPMD
     program serves all; the nc is built (and cached) per
     span-structure — exact for the given inputs, correct for any.
  2. HOST ships per-chunk span bounds (s, e-1 shifted by the chunk's
     first block) as int16, replicated across partitions: 0.5 MB.
  3. DEVICE: per window (chunk j, block b), the token-major indicator
        MT[t, i] = (s16[i] <= tg) * (e16m1[i] >= tg),  tg = t + 128*b'
     builds with two fused DVE compares against a per-window column of
     the block-shifted iota table — no PE transposes, no gathers.
     Per chunk, K_j matmuls accumulate
        out_j += MT.T @ x_block        (f16, f32 PSUM)
     chasing the f32->f16 casts of the streamed sequence.
  4. Scale rows by 1/w during the PSUM->SBUF copy (scalar engine,
     activation scale), store contiguous; HOST unpermutes rows.

Precision: binary f16 indicator is exact; x quantized to f16 (2^-11)
=> ~2e-4 global rel err.
"""

import numpy as np

B, S, D = 8, 4096, 256
N_SPANS = 1024
P = 128
NBLK = S // P
JG = N_SPANS // P      # 8 span chunks of 128

_cache = {"key": None}


def _plan_windows(si):
    """Sorted-span chunk block windows, unioned across cores."""
    perms = np.empty((B, N_SPANS), dtype=np.int64)
    ss = np.empty((B, N_SPANS), dtype=np.int64)
    ee = np.empty((B, N_SPANS), dtype=np.int64)
    for b in range(B):
        perm = np.argsort(si[b, :, 0], kind="stable")
        perms[b] = perm
        ss[b] = si[b, perm, 0]
        ee[b] = si[b, perm, 1]
    windows = []
    for j in range(JG):
        b0 = NBLK
        b1 = 0
        for b in range(B):
            cs = ss[b, j * P : (j + 1) * P]
            ce = ee[b, j * P : (j + 1) * P]
            b0 = min(b0, int(cs.min()) >> 7)
            b1 = max(b1, (int(ce.max()) - 1) >> 7)
        windows.append((b0, b1 - b0 + 1))
    return perms, windows, ss, ee


def build_nc(windows):
    import concourse.bacc as bacc
    import concourse.mybir as mybir
    from concourse.tile import TileContext

    f32 = mybir.dt.float32
    f16 = mybir.dt.float16
    i16 = mybir.dt.int16
    i32 = mybir.dt.int32
    Alu = mybir.AluOpType
    Act = mybir.ActivationFunctionType

    KMAX = max(k for _, k in windows)

    nc = bacc.Bacc(None, target_bir_lowering=False, debug=False, num_devices=B)
    seq = nc.declare_dram_parameter("seq", [S, D], f32, isOutput=False)
    # per-chunk block-shifted bounds (s, e-1), replicated across partitions
    scd = nc.declare_dram_parameter("scd", [P, N_SPANS], i16, isOutput=False)
    ecd = nc.declare_dram_parameter("ecd", [P, N_SPANS], i16, isOutput=False)
    wrec = nc.declare_dram_parameter("wrec", [P, JG], f32, isOutput=False)
    out = nc.declare_dram_parameter("out", [N_SPANS, D], f32, isOutput=True)

    wbase = []
    w0 = 0
    for j in range(JG):
        wbase.append(w0)
        w0 += windows[j][1]
    NW = w0

    with TileContext(nc) as tc:
        with (
            tc.tile_pool(name="const", bufs=1) as const_pool,
            tc.tile_pool(name="x", bufs=4) as x_pool,
            tc.tile_pool(name="a", bufs=4) as a_pool,
            tc.tile_pool(name="ps", bufs=4, space="PSUM") as ps_pool,
            tc.tile_pool(name="misc", bufs=1) as misc_pool,
            tc.tile_pool(name="res", bufs=3) as res_pool,
        ):
            # TB[p, c] = p + 128*c  (token id of row p in window-block c)
            tbi = const_pool.tile([P, KMAX], i32)
            nc.gpsimd.iota(
                tbi[:], pattern=[[P, KMAX]], base=0, channel_multiplier=1
            )
            TB = const_pool.tile([P, KMAX], f32)
            nc.gpsimd.tensor_copy(out=TB[:], in_=tbi[:])

            SC = misc_pool.tile([P, N_SPANS], i16)
            EC = misc_pool.tile([P, N_SPANS], i16)
            # chunk 0/1 bounds land before the first seq group; the rest
            # follow the second group
            for j in range(2):
                nc.sync.dma_start(
                    out=SC[:, j * P : (j + 1) * P], in_=scd[:, j * P : (j + 1) * P]
                )
                nc.sync.dma_start(
                    out=EC[:, j * P : (j + 1) * P], in_=ecd[:, j * P : (j + 1) * P]
                )
            WR = misc_pool.tile([P, JG], f32)
            nc.scalar.dma_start(out=WR[:], in_=wrec[:])

            # first two groups are small so the PE starts early
            GSIZES = [2, 2] + [4] * ((NBLK - 4) // 4)
            NG = len(GSIZES)
            GOFF = [sum(GSIZES[:g]) for g in range(NG)]
            XH = misc_pool.tile([P, NBLK * D], f16)
            MTbig = misc_pool.tile([P, NW, P], f16)

            bigxs = [None] * NG

            def emit_load(g):
                t0 = GOFF[g] * P
                gb = GSIZES[g]
                bigx = x_pool.tile([P, gb * D], f32, name=f"bigx{g}")
                nc.sync.dma_start(
                    out=bigx[:],
                    in_=seq[t0 : t0 + gb * P, :].rearrange(
                        "(m p) d -> p m d", p=P
                    ),
                )
                bigxs[g] = bigx

            emit_load(0)
            emit_load(1)
            nc.sync.dma_start(out=SC[:, 2 * P :], in_=scd[:, 2 * P :])
            nc.sync.dma_start(out=EC[:, 2 * P :], in_=ecd[:, 2 * P :])
            for g in range(2, NG):
                emit_load(g)

            def emit_cast(g):
                lo = GOFF[g] * D
                xsl = XH[:, lo : lo + GSIZES[g] * D]
                if g % 2 == 0:
                    nc.vector.tensor_copy(out=xsl, in_=bigxs[g][:])
                else:
                    nc.scalar.activation(out=xsl, in_=bigxs[g][:], func=Act.Copy)

            def emit_indicators(j):
                b0, kj = windows[j]
                sj = SC[:, j * P : (j + 1) * P].rearrange(
                    "p (one t) -> p one t", one=1
                ).to_broadcast([P, kj, P])
                ej = EC[:, j * P : (j + 1) * P].rearrange(
                    "p (one t) -> p one t", one=1
                ).to_broadcast([P, kj, P])
                tb = TB[:, 0:kj].to_broadcast([P, kj, P])
                At = a_pool.tile([P, KMAX, P], f16, name=f"At{j % 2}")
                A = At[:, 0:kj, :]
                nc.vector.tensor_tensor(out=A, in0=tb, in1=sj, op=Alu.is_ge)
                Btf = a_pool.tile([P, KMAX, P], f16, name=f"Btf{j % 2}")
                Bt = Btf[:, 0:kj, :]
                nc.vector.tensor_tensor(out=Bt, in0=ej, in1=tb, op=Alu.is_ge)
                nc.vector.tensor_tensor(
                    out=MTbig[:, wbase[j] : wbase[j] + kj, :],
                    in0=A, in1=Bt, op=Alu.mult,
                )

            def emit_mms(j):
                b0, kj = windows[j]
                ps = ps_pool.tile([P, D], f32)
                for bb in range(kj):
                    blk = b0 + bb
                    nc.tensor.matmul(
                        out=ps[:],
                        lhsT=MTbig[:, wbase[j] + bb, :],
                        rhs=XH[:, blk * D : (blk + 1) * D],
                        start=(bb == 0), stop=(bb == kj - 1),
                    )
                rj = res_pool.tile([P, D], f32)
                nc.scalar.activation(
                    out=rj[:], in_=ps[:], func=Act.Copy,
                    scale=WR[:, j : j + 1],
                )
                oj = out[:].rearrange("(c p) d -> p c d", p=P)[:, j, :]
                nc.scalar.dma_start(out=oj, in_=rj[:])

            emit_cast(0)
            emit_cast(1)
            emit_indicators(0)
            for j in range(JG):
                if j + 2 < NG:
                    emit_cast(j + 2)
                if j + 1 < JG:
                    emit_indicators(j + 1)
                emit_mms(j)
    nc.finalize()
    return nc


def _make_in_maps(sequence_tensor, si, perms, windows, ss, ee):
    seq = np.ascontiguousarray(np.asarray(sequence_tensor), dtype=np.float32)
    in_maps = []
    for b in range(B):
        sc = np.empty(N_SPANS, dtype=np.int16)
        ec = np.empty(N_SPANS, dtype=np.int16)
        for j in range(JG):
            b0 = windows[j][0]
            sl = slice(j * P, (j + 1) * P)
            sc[sl] = ss[b, sl] - 128 * b0
            ec[sl] = ee[b, sl] - 1 - 128 * b0
        wr = (
            1.0 / (ee[b] - ss[b]).astype(np.float32)
        ).reshape(JG, P).T.copy()
        in_maps.append(
            {
                "seq": seq[b],
                "scd": np.tile(sc, (P, 1)),
                "ecd": np.tile(ec, (P, 1)),
                "wrec": wr,
            }
        )
    return in_maps


def kernel(sequence_tensor, span_indices):
    from concourse.bass_utils import run_bass_kernel_spmd

    si = np.asarray(span_indices)
    assert si.shape == (B, N_SPANS, 2)
    key = si.tobytes()
    if _cache["key"] != key:
        perms, windows, ss, ee = _plan_windows(si)
        _cache.update(
            key=key, nc=build_nc(windows),
            plan=(perms, windows, ss, ee),
        )
    perms, windows, ss, ee = _cache["plan"]
    in_maps = _make_in_maps(sequence_tensor, si, perms, windows, ss, ee)
    res = run_bass_kernel_spmd(_cache["nc"], in_maps, list(range(B)))
    full = np.empty((B, N_SPANS, D), dtype=np.float32)
    for b in range(B):
        full[b, perms[b], :] = res.results[b]["out"]
    return full
